# revision 29
# baseline (speedup 1.0000x reference)
"""GCN 2-layer kernel for Trainium2 (8 NeuronCores, Bass/Tile).

Strategy (v3 — PE-matmul scatter, no dma_scatter_add):
  - Nodes snake-dealt into 8 cores by degree, then a per-core greedy
    balances (src-range x dst-tile) edge counts across the 104 tiles
    (SPMD needs static chunk counts = max over cores; padding ~6.5%).
  - Global gather table has a 16384-row stripe per core (13312 real +
    zero pad), so each 32768-row int16 index range = exactly 2 stripes
    and a src's range is known from its core alone.
  - Per layer: g = dis * (h @ W) lands in an SBUF accumulator (the
    self-loop term) and in a bounce buffer; AllGather builds the full
    fp32 g-table [131072, 64] in DRAM on every core (collectives are
    cheap on this fabric).
  - Edge phase: edges sorted by (range, dst-tile), each group padded to
    a multiple of 128 tokens (pads point at a known zero row).
    dma_gather fetches 1024-token blocks (1 descriptor per edge, SWDGE
    ring cap, 4 queues round-robin); the index table stays SBUF-resident.
    Act converts each block fp32->bf16. One DVE tensor_tensor per block
    builds all 8 one-hot S matrices at once via stride-0 broadcast APs
    (is_equal(iota_row, dst_slot)). Each 128-token chunk is then
    scatter-added into its destination tile by a PE matmul
    psum[128 dst, 64] += S^T @ msgs (bf16, FWL), accumulated per
    (range, tile) group in PSUM and added into the SBUF accumulator.
  - Tails: z = relu(dis*acc + b1) -> z @ W2 -> g2 (same structure);
    layer 2 ends with log_softmax. ~1.2 ms/iter vs 3.78 ms baseline.
"""
import os
import sys

sys.path.insert(0, "/opt/trn_rl_repo")

import numpy as np

N, FIN, H, C = 100000, 128, 64, 64
E = 1600000
NCORES = 8
NT = 104                  # tiles per core
S = NT * 128              # 13312 slots per core
NG8 = NT // 8             # 13 groups of 8 tiles
SCORE = 16384             # table stripe per core (13312 real + pad)
GROWS = NCORES * SCORE    # 131072 global table rows
RANGE_W = 32768           # int16-addressable row window = 2 core stripes
NRANGES = GROWS // RANGE_W                   # 4
ZREL = S                  # known-zero row (rel) in every range
GB = int(os.environ.get("KGB", "1024"))   # tokens per gather instruction
                                          # (<= SWDGE ring = scratch/16)
NQ = 4                    # SWDGE queues


def _plan(x, edge_index):
    """Host-side planning. Returns per-core input arrays + chunk schedule."""
    x = np.asarray(x, np.float32)
    ei = np.asarray(edge_index, np.int64)
    src, dst = ei[0], ei[1]

    deg = np.bincount(dst, minlength=N).astype(np.float64) + 1.0  # + self loop
    dis = (1.0 / np.sqrt(deg)).astype(np.float32)

    # snake deal into cores by degree desc (balance per-core degree mass)
    order = np.argsort(-deg, kind="stable")
    pos = np.arange(N)
    blk, rem = pos // NCORES, pos % NCORES
    corepat = np.where(blk % 2 == 0, rem, NCORES - 1 - rem)
    core_of = np.empty(N, np.int64)
    core_of[order] = corepat

    # per-node in-degree split by src range (= src core pair, since each
    # 32768-row range is exactly 2 core stripes)
    dr = np.zeros((N, NRANGES), np.int64)
    np.add.at(dr, (dst, core_of[src] // 2), 1)

    # per-core greedy: nodes by degree desc -> tile minimizing dot(cnt, d)
    # (quadratic-balance surrogate), capacity 128 slots per tile
    tile_of = np.empty(N, np.int64)
    slot_of = np.empty(N, np.int64)
    BIG = 1 << 40
    for c in range(NCORES):
        nodes = order[core_of[order] == c]
        cnt = np.zeros((NT, NRANGES), np.int64)
        slots = np.zeros(NT, np.int64)
        for n in nodes:
            d = dr[n]
            cost = cnt @ d + np.where(slots >= 128, BIG, 0)
            t = int(np.argmin(cost))
            tile_of[n] = t
            slot_of[n] = slots[t]
            slots[t] += 1
            cnt[t] += d
        assert slots.max() <= 128

    # local row scramble (DMA-friendly layout): row = 1024*(t//8) + 8*s + (t%8)
    lrow_of = 1024 * (tile_of // 8) + 8 * slot_of + (tile_of % 8)
    grow_of = core_of * SCORE + lrow_of
    zrel = np.full(NRANGES, ZREL, np.int64)   # stripe pad row, zeroed on dev

    # per-core edge lists keyed by (src range, dst tile)
    src_g = grow_of[src]
    er_all = core_of[src] // 2
    dst_c = core_of[dst]
    NGRP = NRANGES * NT
    counts = np.zeros((NCORES, NGRP), np.int64)
    per_core = []
    sort_src = os.environ.get("KSORTSRC", "1") == "1"
    for c in range(NCORES):
        m = dst_c == c
        key = er_all[m] * NT + tile_of[dst[m]]
        if sort_src:
            # ascending src within each group -> monotonic HBM addresses
            six = np.lexsort((src_g[m], key))
        else:
            six = np.argsort(key, kind="stable")
        per_core.append((key[six],
                         (src_g[m] - er_all[m] * RANGE_W)[six],
                         slot_of[dst[m]][six]))
        counts[c] = np.bincount(key, minlength=NGRP)

    # static chunks per group = max over cores
    C_g = (counts.max(axis=0) + 127) // 128           # [NGRP]
    span_g = 128 * C_g
    tok_start = np.concatenate([[0], np.cumsum(span_g)])
    TOKTOT = int(tok_start[-1])
    NCH = TOKTOT // 128

    # chunk metadata (static across cores)
    grp_of_chunk = np.repeat(np.arange(NGRP), C_g)
    ch_r = grp_of_chunk // NT
    ch_t = grp_of_chunk % NT
    ch_first = np.r_[True, grp_of_chunk[1:] != grp_of_chunk[:-1]]
    ch_last = np.r_[grp_of_chunk[1:] != grp_of_chunk[:-1], True]

    # per-range token spans and gather blocks
    rng_tok = np.zeros(NRANGES + 1, np.int64)
    for r in range(NRANGES):
        rng_tok[r + 1] = rng_tok[r] + int(span_g[r * NT:(r + 1) * NT].sum())
    blocks = []   # (r, tok_off, ntok)
    for r in range(NRANGES):
        o = int(rng_tok[r])
        while o < rng_tok[r + 1]:
            nt_ = int(min(GB, rng_tok[r + 1] - o))
            blocks.append((r, o, nt_))
            o += nt_

    # per-core token arrays
    gidx_all = np.zeros((NCORES, 128, TOKTOT // 16), np.int16)
    dcols_all = np.zeros((NCORES, 128, NCH), np.float32)
    for c in range(NCORES):
        key_s, rel_s, ds_s = per_core[c]
        # pad value per token = zrel of its group's range
        gtok = np.repeat(zrel[np.arange(NGRP) // NT], span_g)
        # positions of real edges in the token stream
        gstart_sorted = np.searchsorted(key_s, np.arange(NGRP))
        within = np.arange(len(key_s)) - gstart_sorted[key_s]
        tpos = tok_start[key_s] + within
        gtok[tpos] = rel_s
        dtok = np.zeros(TOKTOT, np.int64)
        dtok[tpos] = ds_s
        gidx_all[c] = np.tile(
            gtok.astype(np.int16).reshape(TOKTOT // 16, 16).T, (8, 1))
        dcols_all[c] = dtok.reshape(NCH, 128).T.astype(np.float32)

    # per-core xT / dis arranged by (tile, partition) scramble
    lin = np.arange(S)
    tile_id = 8 * (lin // 1024) + lin % 8
    part_id = (lin % 1024) // 8
    col = tile_id * 128 + part_id

    xT_all = np.zeros((NCORES, 128, S), np.float32)
    dis_all = np.ones((NCORES, 128, NT), np.float32)
    nodes_by_core = []
    for c in range(NCORES):
        mc = core_of == c
        nodes_c = np.flatnonzero(mc)
        nodes_by_core.append(nodes_c)
        lr = lrow_of[nodes_c]
        xT_all[c][:, col[lr]] = x[nodes_c].T
        dis_all[c][slot_of[nodes_c], tile_of[nodes_c]] = dis[nodes_c]

    return {
        "TOKTOT": TOKTOT, "NCH": NCH,
        "ch_r": ch_r, "ch_t": ch_t,
        "ch_first": ch_first, "ch_last": ch_last,
        "blocks": blocks,
        "gidx": gidx_all, "dcols": dcols_all,
        "xT": xT_all, "dis": dis_all,
        "nodes_by_core": nodes_by_core,
        "lrow_of": lrow_of,
    }


def _build(plan, stage=99):
    import concourse.bacc as bacc
    import concourse.bass as bass
    import concourse.tile as tile
    import concourse.mybir as mybir
    from concourse.masks import make_identity

    f32 = mybir.dt.float32
    bf16 = mybir.dt.bfloat16
    i16 = mybir.dt.int16
    AF = mybir.ActivationFunctionType
    ALU = mybir.AluOpType

    TOKTOT = plan["TOKTOT"]
    NCH = plan["NCH"]
    ch_r, ch_t = plan["ch_r"], plan["ch_t"]
    ch_first, ch_last = plan["ch_first"], plan["ch_last"]
    blocks = plan["blocks"]

    scratch = max(16384, 16 * GB)
    nc = bacc.Bacc("TRN2", target_bir_lowering=False, debug=False,
                   num_devices=NCORES, num_swdge_queues=NQ,
                   dynamic_dma_scratch_size=scratch)

    t_xT = nc.dram_tensor("xT", [128, S], f32, kind="ExternalInput")
    t_dis = nc.dram_tensor("dis", [128, NT], f32, kind="ExternalInput")
    t_W1 = nc.dram_tensor("W1", [FIN, H], f32, kind="ExternalInput")
    t_W2 = nc.dram_tensor("W2", [H, C], f32, kind="ExternalInput")
    t_b1 = nc.dram_tensor("b1b", [128, H], f32, kind="ExternalInput")
    t_b2 = nc.dram_tensor("b2b", [128, C], f32, kind="ExternalInput")
    t_io = nc.dram_tensor("iota128", [128, 128], bf16, kind="ExternalInput")
    t_gi = nc.dram_tensor("gidx", [128, TOKTOT // 16], i16,
                          kind="ExternalInput")
    t_dc = nc.dram_tensor("dcols", [128, NCH], bf16,
                          kind="ExternalInput")
    t_y = nc.dram_tensor("y", [S, C], f32, kind="ExternalOutput")

    tspace = "Shared" if os.environ.get("KSHARED", "0") == "1" else "Local"
    g1_bounce = nc.dram_tensor("g1_bounce", [SCORE, H], f32, kind="Internal")
    g2_bounce = nc.dram_tensor("g2_bounce", [SCORE, C], f32, kind="Internal")
    g1_table = nc.dram_tensor("g1_table", [GROWS, H], f32, kind="Internal",
                              addr_space=tspace)
    g2_table = nc.dram_tensor("g2_table", [GROWS, C], f32, kind="Internal",
                              addr_space=tspace)

    with tile.TileContext(nc) as tc:
        NBUF = int(os.environ.get("KNBUF", "8"))
        with tc.tile_pool(name="sb", bufs=1) as sbc, \
             tc.tile_pool(name="sbw", bufs=3) as sb, \
             tc.tile_pool(name="sbg", bufs=3) as sbg, \
             tc.tile_pool(name="sbe", bufs=NBUF) as sbe, \
             tc.tile_pool(name="sbs", bufs=4) as sbs, \
             tc.tile_pool(name="psA", bufs=int(os.environ.get("KPSA", "4")),
                          space="PSUM") as psA, \
             tc.tile_pool(name="psT", bufs=int(os.environ.get("KPST", "2")),
                          space="PSUM") as psT, \
             tc.tile_pool(name="psN", bufs=int(os.environ.get("KPSN", "2")),
                          space="PSUM") as psN:

            W1t = sbc.tile([FIN, H], f32)
            nc.sync.dma_start(out=W1t[:], in_=t_W1[:])
            W2t = sbc.tile([H, C], f32)
            nc.sync.dma_start(out=W2t[:], in_=t_W2[:])
            b1t = sbc.tile([128, H], f32)
            nc.sync.dma_start(out=b1t[:], in_=t_b1[:])
            b2t = sbc.tile([128, C], f32)
            nc.sync.dma_start(out=b2t[:], in_=t_b2[:])
            iot = sbc.tile([128, 128], bf16)
            nc.sync.dma_start(out=iot[:], in_=t_io[:])
            dis = sbc.tile([128, NT], f32)
            nc.sync.dma_start(out=dis[:], in_=t_dis[:])
            dct = sbc.tile([128, NCH], bf16)
            nc.sync.dma_start(out=dct[:], in_=t_dc[:])
            git = sbc.tile([128, TOKTOT // 16], i16)
            nc.sync.dma_start(out=git[:], in_=t_gi[:])
            ident = sbc.tile([128, 128], f32)
            make_identity(nc, ident[:])
            zrow = sbc.tile([1, 128], f32)
            nc.vector.memset(zrow[:], 0.0)

            acc1 = sbc.tile([128, NT, H], f32)
            acc2 = sbc.tile([128, NT, C], f32)

            KREP = int(os.environ.get("KREP", "1"))
            for _rep in range(KREP):
                # -------- prep: acc1 tiles = g1 = dis * (x @ W1) ----------
                for Gi in range(NG8 if stage >= 1 else 0):
                    xc = sbg.tile([128, 1024], f32, tag="xc")
                    nc.sync.dma_start(out=xc[:],
                                      in_=t_xT[:, Gi * 1024:(Gi + 1) * 1024])
                    for j in range(8):
                        t = 8 * Gi + j
                        h1T = psT.tile([H, 128], f32, tag="pT")
                        nc.tensor.matmul(out=h1T[:], lhsT=W1t[:],
                                         rhs=xc[:, j * 128:(j + 1) * 128],
                                         start=True, stop=True)
                        h1Ts = sb.tile([H, 128], f32, tag="hTs")
                        nc.scalar.copy(h1Ts[:], h1T[:])
                        h1 = psN.tile([128, H], f32, tag="pN")
                        nc.tensor.transpose(out=h1[:], in_=h1Ts[:],
                                            identity=ident[0:H, 0:H])
                        nc.vector.tensor_scalar(
                            out=acc1[:, t, :], in0=h1[:],
                            scalar1=dis[:, t:t + 1], scalar2=None,
                            op0=ALU.mult)
                    nc.sync.dma_start(
                        out=g1_bounce[Gi * 1024:(Gi + 1) * 1024, :],
                        in_=acc1[:, 8 * Gi:8 * Gi + 8, :])

                if stage >= 2:
                    nc.sync.dma_start(out=g1_bounce[ZREL:ZREL + 1, :],
                                      in_=zrow[:, 0:H])
                    nc.gpsimd.collective_compute(
                        "AllGather", mybir.AluOpType.bypass,
                        replica_groups=[list(range(NCORES))],
                        ins=[g1_bounce[:]], outs=[g1_table[:]])

                # ---------------- edge phase ----------------
                emode = os.environ.get("KEMODE", "full")
                knq = int(os.environ.get("KNQ", str(NQ)))
                ksp = os.environ.get("KSP", "0") == "1"
                kelem = int(os.environ.get("KELEM", str(H)))
                BPC = GB // 128   # chunks per full block

                def edge_phase(table, acc, ln):
                    ch = 0
                    qi = 0
                    cur_pt = None
                    for (r, toff, ntok) in blocks:
                        r0 = r * RANGE_W
                        r1 = min((r + 1) * RANGE_W, GROWS)
                        nblk = ntok // 128
                        buf = sbe.tile([128, BPC, H], f32, tag="gbuf")
                        if emode != "conly":
                            gi = git[:, toff // 16:(toff + ntok) // 16]
                            if kelem == H:
                                in_ap = table[r0:r1, :]
                                out_ap = buf[:, 0:nblk, :]
                            else:
                                # timing experiment: larger elems, bogus data
                                tv = table[:].rearrange(
                                    "(a b) h -> a (b h)", b=kelem // H)
                                in_ap = tv[0:RANGE_W, :]
                                bufw = sbe.tile([128, BPC, kelem], f32,
                                                tag="gbufw")
                                out_ap = bufw[:, 0:nblk, :]
                            nc.gpsimd.dma_gather(
                                out_ap=out_ap,
                                in_ap=in_ap,
                                idxs_ap=gi,
                                num_idxs=ntok,
                                num_idxs_reg=ntok,
                                elem_size=kelem,
                                queue_num=qi % knq,
                                single_packet=ksp,
                            )
                        else:
                            nc.vector.memset(buf[:, 0:nblk, :], 0.0)
                        qi += 1
                        if emode == "gonly":
                            ch += nblk
                            continue
                        bufb = sbe.tile([128, BPC, H], bf16, tag="gbufb")
                        nc.scalar.copy(bufb[:, 0:nblk, :],
                                       buf[:, 0:nblk, :])
                        # batched one-hot build: all chunks of the block in
                        # one DVE op via stride-0 broadcast APs
                        S8 = sbe.tile([128, BPC, 128], bf16, tag="S8")
                        nc.vector.tensor_tensor(
                            out=S8[:, 0:nblk, :],
                            in0=iot[:].unsqueeze(1).broadcast_to(
                                [128, nblk, 128]),
                            in1=dct[:, ch:ch + nblk].unsqueeze(2).broadcast_to(
                                [128, nblk, 128]),
                            op=ALU.is_equal)
                        for k in range(nblk):
                            t = int(ch_t[ch])
                            if ch_first[ch]:
                                cur_pt = psA.tile([128, H], f32, tag="pacc")
                            nc.tensor.matmul(
                                out=cur_pt[:], lhsT=S8[:, k, :],
                                rhs=bufb[:, k, :],
                                start=bool(ch_first[ch]),
                                stop=bool(ch_last[ch]))
                            if ch_last[ch]:
                                nc.vector.tensor_tensor(
                                    out=acc[:, t, :], in0=acc[:, t, :],
                                    in1=cur_pt[:], op=ALU.add)
                            ch += 1
                    assert ch == NCH

                if stage >= 3:
                    edge_phase(g1_table, acc1, 1)

                # ------------- layer-1 tails: z=relu(dis*s+b1); g2=dis*(z@W2)
                if stage >= 4:
                    for Gi in range(NG8):
                        for j in range(8):
                            t = 8 * Gi + j
                            zp = sb.tile([128, H], f32, tag="zp")
                            nc.vector.tensor_scalar(
                                out=zp[:], in0=acc1[:, t, :],
                                scalar1=dis[:, t:t + 1],
                                scalar2=None, op0=ALU.mult)
                            nc.vector.tensor_tensor(out=zp[:], in0=zp[:],
                                                    in1=b1t[:], op=ALU.add)
                            z = sb.tile([128, H], f32, tag="z")
                            nc.scalar.activation(z[:], zp[:], AF.Relu)
                            zT = psT.tile([H, 128], f32, tag="pT")
                            nc.tensor.transpose(out=zT[:], in_=z[:],
                                                identity=ident[:])
                            zTs = sb.tile([H, 128], f32, tag="zTs")
                            nc.scalar.copy(zTs[:], zT[:])
                            h2T = psT.tile([C, 128], f32, tag="pT")
                            nc.tensor.matmul(out=h2T[:], lhsT=W2t[:],
                                             rhs=zTs[:], start=True, stop=True)
                            h2Ts = sb.tile([C, 128], f32, tag="h2Ts")
                            nc.scalar.copy(h2Ts[:], h2T[:])
                            h2 = psN.tile([128, C], f32, tag="pN")
                            nc.tensor.transpose(out=h2[:], in_=h2Ts[:],
                                                identity=ident[0:C, 0:C])
                            nc.vector.tensor_scalar(
                                out=acc2[:, t, :], in0=h2[:],
                                scalar1=dis[:, t:t + 1], scalar2=None,
                                op0=ALU.mult)
                        nc.sync.dma_start(
                            out=g2_bounce[Gi * 1024:(Gi + 1) * 1024, :],
                            in_=acc2[:, 8 * Gi:8 * Gi + 8, :])

                if stage >= 5:
                    nc.sync.dma_start(out=g2_bounce[ZREL:ZREL + 1, :],
                                      in_=zrow[:, 0:C])
                    nc.gpsimd.collective_compute(
                        "AllGather", mybir.AluOpType.bypass,
                        replica_groups=[list(range(NCORES))],
                        ins=[g2_bounce[:]], outs=[g2_table[:]])

                if stage >= 6:
                    edge_phase(g2_table, acc2, 2)

                # ---------------- layer-2 tails: log_softmax ----------------
                for Gi in range(NG8 if stage >= 6 else 0):
                    y8 = sbg.tile([128, 8, C], f32, tag="y8")
                    for j in range(8):
                        t = 8 * Gi + j
                        lg = sb.tile([128, C], f32, tag="lg")
                        nc.vector.tensor_scalar(
                            out=lg[:], in0=acc2[:, t, :],
                            scalar1=dis[:, t:t + 1], scalar2=None,
                            op0=ALU.mult)
                        nc.vector.tensor_tensor(out=lg[:], in0=lg[:],
                                                in1=b2t[:], op=ALU.add)
                        nmax = sb.tile([128, 1], f32, tag="nmax")
                        nc.vector.tensor_reduce(
                            out=nmax[:], in_=lg[:], axis=mybir.AxisListType.X,
                            op=ALU.max, negate=True)
                        ex = sb.tile([128, C], f32, tag="ex")
                        sume = sb.tile([128, 1], f32, tag="sume")
                        nc.scalar.activation(ex[:], lg[:], AF.Exp,
                                             bias=nmax[:], scale=1.0,
                                             accum_out=sume[:])
                        lse = sb.tile([128, 1], f32, tag="lse")
                        nc.scalar.activation(lse[:], sume[:], AF.Ln)
                        cc = sb.tile([128, 1], f32, tag="cc")
                        nc.vector.tensor_tensor(out=cc[:], in0=nmax[:],
                                                in1=lse[:], op=ALU.subtract)
                        nc.vector.tensor_scalar(
                            out=y8[:, j, :], in0=lg[:], scalar1=cc[:],
                            scalar2=None, op0=ALU.add)
                    nc.sync.dma_start(
                        out=t_y[Gi * 1024:(Gi + 1) * 1024, :], in_=y8[:])

    nc.compile()
    return nc


def _run(inputs, trace=False):
    import concourse.bass_utils as bass_utils

    x = np.asarray(inputs["x"], np.float32)
    W1 = np.asarray(inputs["W1"], np.float32)
    b1 = np.asarray(inputs["b1"], np.float32)
    W2 = np.asarray(inputs["W2"], np.float32)
    b2 = np.asarray(inputs["b2"], np.float32)

    plan = _plan(x, inputs["edge_index"])
    nc = _build(plan, stage=int(os.environ.get("KSTAGE", "99")))

    b1b = np.tile(b1[None, :], (128, 1)).astype(np.float32)
    b2b = np.tile(b2[None, :], (128, 1)).astype(np.float32)
    iota128 = np.tile(np.arange(128, dtype=np.float32)[None, :], (128, 1))
    import ml_dtypes
    iota128 = iota128.astype(ml_dtypes.bfloat16)

    in_maps = []
    for c in range(NCORES):
        in_maps.append({
            "xT": plan["xT"][c],
            "dis": plan["dis"][c],
            "W1": W1, "W2": W2, "b1b": b1b, "b2b": b2b,
            "iota128": iota128,
            "gidx": plan["gidx"][c],
            "dcols": plan["dcols"][c].astype(ml_dtypes.bfloat16),
        })

    res = bass_utils.run_bass_kernel_spmd(
        nc, in_maps, core_ids=list(range(NCORES)), trace=trace)

    out = np.empty((N, C), np.float32)
    for c in range(NCORES):
        yc = np.asarray(res.results[c]["y"], np.float32)
        nodes_c = plan["nodes_by_core"][c]
        out[nodes_c] = yc[plan["lrow_of"][nodes_c]]
    return out, res


def kernel(**inputs):
    out, _ = _run(inputs, trace=False)
    return out
